# revision 33
# baseline (speedup 1.0000x reference)
"""DeepSeek-V3 MLA attention (B=1, S=1024, D=7168, H=128) on 8 Trainium2
NeuronCores.

Sharding: tensor-parallel over the 128 heads (16 heads/core) for
q_b/kv_b/attention; the small latent projections (wq_a, wkv_a) are
M-sharded (256 rows/core) with ONE fused AllGather of the raw latents;
k_pe is K-sharded and combined with one small AllReduce; head outputs
are AllGathered (bf16) in 8 chunks of 2 heads, and o_proj is row-sharded
(896 output features/core) with its K-accumulation interleaved into the
attention head loop (per-AG-chunk sub-passes accumulating into an SBUF
fp32 accumulator) so the tensor engine never idles.

rmsnorm is applied by normalizing the latents in SBUF *after* the
sum-of-squares reductions, overlapped with the kv_b matmul stream
(k_nope consumes raw latents and is scaled at PSUM evacuation).

All activations are feature-major [feat, token]; matmuls run in bf16
with fp32 PSUM accumulation; softmax runs without max-subtraction
(scores are O(5)) with exp in fp32->bf16 and exact causal masking via a
0/1 triangular mask on the diagonal tiles.
"""

import os
from contextlib import ExitStack

import numpy as np
import ml_dtypes

import concourse.bass as bass
import concourse.mybir as mybir
import concourse.tile as tile
from concourse import bacc
from concourse.bass_utils import run_bass_kernel_spmd
from concourse.masks import make_upper_triangular

bf16 = ml_dtypes.bfloat16
F32 = mybir.dt.float32
BF = mybir.dt.bfloat16

B, S, D = 1, 1024, 7168
H, DN, DR, DV = 128, 128, 64, 128
DQ = DN + DR                  # 192
RQ, RKV = 1536, 512
EPS = 1e-6
SCALE = float(DQ) ** -0.5
NC = 8
HC = H // NC                  # 16 heads per core
T = S
KT_X = D // 128               # 56
KT_Q = RQ // 128              # 12
KT_KV = RKV // 128            # 4
MT_QB = (HC * DQ) // 128      # 24 (16 nope tiles + 8 rope tiles)
NAG = 16                      # attention-output AllGather chunks (1 head each)
HPC = HC // NAG               # heads per AG chunk = 1
KTC = H * DV // 128 // NAG    # o_proj k-tiles per chunk = 8
DEBUG = bool(int(os.environ.get("BASSMLA_DEBUG", "0")))

_CACHE = {}


def _build():
    nc = bacc.Bacc("TRN2", target_bir_lowering=False, debug=False, num_devices=NC)

    x_in = nc.dram_tensor("x", [128, KT_X, T], BF, kind="ExternalInput").ap()
    xkpe_in = nc.dram_tensor("xkpe", [128, 7, T], BF, kind="ExternalInput").ap()
    wa_in = nc.dram_tensor("wa", [128, KT_X, 2, 128], BF, kind="ExternalInput").ap()
    wkpe_in = nc.dram_tensor("wkpe", [128, 7, DR], BF, kind="ExternalInput").ap()
    ccss_in = nc.dram_tensor("ccss", [128, 2 * T], F32, kind="ExternalInput").ap()
    wqb_in = nc.dram_tensor("wqb", [128, MT_QB, KT_Q, 128], BF, kind="ExternalInput").ap()
    wkn_in = nc.dram_tensor("wkn", [128, HC, KT_KV, 128], BF, kind="ExternalInput").ap()
    wv_in = nc.dram_tensor("wv", [128, KT_KV, HC * DV], BF, kind="ExternalInput").ap()
    # [128, 7(m), 128(ktg), 128] so a (m, ktg-range) slice is contiguous
    wo_in = nc.dram_tensor("wo", [128, 7, H * DV // 128, 128], BF, kind="ExternalInput").ap()
    out_ap = nc.dram_tensor("out", [D // NC, T], BF, kind="ExternalOutput").ap()
    if DEBUG:
        dbg_ckv = nc.dram_tensor("dbg_ckv", [RKV, T], BF, kind="ExternalOutput").ap()
        dbg_qn = nc.dram_tensor("dbg_qn", [RQ, T], BF, kind="ExternalOutput").ap()
        dbg_kn = nc.dram_tensor("dbg_kn", [128, T], BF, kind="ExternalOutput").ap()
        dbg_v = nc.dram_tensor("dbg_v", [128, HC * DV], BF, kind="ExternalOutput").ap()
        dbg_qh = nc.dram_tensor("dbg_qh", [128, T], BF, kind="ExternalOutput").ap()
        dbg_qr = nc.dram_tensor("dbg_qr", [128, T], BF, kind="ExternalOutput").ap()
        dbg_kp = nc.dram_tensor("dbg_kp", [128, T], BF, kind="ExternalOutput").ap()
        dbg_acc = nc.dram_tensor("dbg_acc", [128, T], BF, kind="ExternalOutput").ap()
        dbg_ob = nc.dram_tensor("dbg_ob", [HC * 128, T], BF, kind="ExternalOutput").ap()

    RG = [list(range(NC))]

    with tile.TileContext(nc) as tc:
        es_dram = ExitStack()
        dram = es_dram.enter_context(tc.tile_pool(name="dram", bufs=1, space="DRAM"))
        ar_in = dram.tile([DR, T], F32, tag="ar_in", name="ar_in")
        db_d = dram.tile([HC, T], BF, tag="db_d", name="db_d")
        ar_out = dram.tile([DR, T], F32, tag="ar_out", name="ar_out", addr_space="Shared")
        agf1_in = dram.tile([128, T], BF, tag="agf1_in", name="agf1_in")
        agf1_out = dram.tile([128 * NC, T], BF, tag="agf1_out", name="agf1_out",
                             addr_space="Shared")
        agf2_in = dram.tile([128, T], BF, tag="agf2_in", name="agf2_in")
        agf2_out = dram.tile([128 * NC, T], BF, tag="agf2_out", name="agf2_out",
                             addr_space="Shared")
        ago_in = [dram.tile([HPC * DV, T], BF, tag=f"ago_in{i}", name=f"ago_in{i}")
                  for i in range(NAG)]
        ago_out = [dram.tile([HPC * DV * NC, T], BF, tag=f"ago_out{i}",
                             name=f"ago_out{i}", addr_space="Shared")
                   for i in range(NAG)]

        es_persist = ExitStack()
        persist = es_persist.enter_context(tc.tile_pool(name="persist", bufs=1))
        ones_bf = persist.tile([128, 1], BF, tag="ones", name="ones")
        nc.vector.memset(ones_bf, 1.0)
        eps_t = persist.tile([1, 1], F32, tag="eps", name="eps")
        nc.vector.memset(eps_t, EPS)
        mask_t = persist.tile([128, 128], BF, tag="mask", name="mask")
        make_upper_triangular(nc, mask_t.opt(), val=1.0, diag=True)
        ccss_bf = persist.tile([128, 2 * T], BF, tag="ccss_bf", name="ccss_bf")
        CCb = ccss_bf[:, 0:T]
        SSb = ccss_bf[:, T:2 * T]
        kroped2 = persist.tile([128, T], BF, tag="kroped2", name="kroped2")

        # ============ Stage A: latent projections (sharded) ============
        es_early = ExitStack()
        early = es_early.enter_context(tc.tile_pool(name="early", bufs=1))
        ccss = early.tile([128, 2 * T], F32, tag="ccss", name="ccss")
        nc.sync.dma_start(out=ccss, in_=ccss_in)
        nc.vector.tensor_copy(ccss_bf, ccss)
        CC = ccss[:, 0:T]
        SS = ccss[:, T:2 * T]
        wkpe_t = early.tile([128, 7, DR], BF, tag="wkpe", name="wkpe")
        nc.sync.dma_start(out=wkpe_t, in_=wkpe_in)

        es_xr = ExitStack()
        xrp = es_xr.enter_context(tc.tile_pool(name="xr", bufs=1))
        es_xpool = ExitStack()
        xpool = es_xpool.enter_context(tc.tile_pool(name="xpool", bufs=4))
        es_psA = ExitStack()
        psA = es_psA.enter_context(tc.tile_pool(name="psA", bufs=1, space="PSUM"))

        # k_pe partials first: the AllReduce fires early and overlaps the rest
        psk = [psA.tile([64, 512], F32, tag=f"k{i}", name=f"k{i}") for i in range(2)]
        for kt in range(7):
            xk = xpool.tile([128, T], BF, tag="xk", name="xk")
            nc.sync.dma_start(out=xk, in_=xkpe_in[:, kt])
            for ch in range(2):
                nc.tensor.matmul(
                    psk[ch], wkpe_t[:, kt, :], xk[:, 512 * ch:512 * (ch + 1)],
                    start=(kt == 0), stop=(kt == 6))
        kpe_loc = early.tile([64, T], F32, tag="kpe_loc", name="kpe_loc")
        for ch in range(2):
            nc.scalar.copy(kpe_loc[:, 512 * ch:512 * (ch + 1)], psk[ch])
        nc.sync.dma_start(out=ar_in[0:64], in_=kpe_loc)

        # two M-passes over resident x: the kv-latent rows (in t1) finish and
        # AllGather first so the kv_b pipeline starts while t0 is still going
        xr = [xrp.tile([128, T], BF, tag=f"x{k}", name=f"x{k}")
              for k in range(KT_X)]
        war = [xrp.tile([128, 2, 128], BF, tag=f"war{k}", name=f"war{k}")
               for k in range(KT_X)]
        psa = [psA.tile([128, 512], F32, tag=f"a{i}", name=f"a{i}") for i in range(4)]
        for kt in range(KT_X):
            nc.sync.dma_start(out=xr[kt], in_=x_in[:, kt])
            nc.sync.dma_start(out=war[kt], in_=wa_in[:, kt])
            for ch in range(2):
                nc.tensor.matmul(
                    psa[2 + ch], war[kt][:, 1, :],
                    xr[kt][:, 512 * ch:512 * (ch + 1)],
                    start=(kt == 0), stop=(kt == KT_X - 1))
        t1_bf = early.tile([128, T], BF, tag="t1_bf", name="t1_bf")
        for ch in range(2):
            nc.scalar.copy(t1_bf[:, 512 * ch:512 * (ch + 1)], psa[2 + ch])
        nc.sync.dma_start(out=agf1_in[:], in_=t1_bf)
        nc.gpsimd.collective_compute(
            "AllGather", mybir.AluOpType.bypass, replica_groups=RG,
            ins=[agf1_in.opt()], outs=[agf1_out.opt()])
        for kt in range(KT_X):
            for ch in range(2):
                nc.tensor.matmul(
                    psa[ch], war[kt][:, 0, :],
                    xr[kt][:, 512 * ch:512 * (ch + 1)],
                    start=(kt == 0), stop=(kt == KT_X - 1))
        t0_bf = early.tile([128, T], BF, tag="t0_bf", name="t0_bf")
        for ch in range(2):
            nc.scalar.copy(t0_bf[:, 512 * ch:512 * (ch + 1)], psa[ch])
        nc.sync.dma_start(out=agf2_in[:], in_=t0_bf)
        nc.gpsimd.collective_compute(
            "AllGather", mybir.AluOpType.bypass, replica_groups=RG,
            ins=[agf2_in.opt()], outs=[agf2_out.opt()])
        # k_pe AllReduce last on the CC queue: its result is only needed by
        # the attention phase, while the AGs gate the kv_b/q_b pipeline
        nc.gpsimd.collective_compute(
            "AllReduce", mybir.AluOpType.add, replica_groups=RG,
            ins=[ar_in.opt()], outs=[ar_out.opt()])

        # k_pe rope (consumes the AllReduce; off the critical path)
        kpe_sb = early.tile([64, T], F32, tag="kpe_sb", name="kpe_sb")
        nc.sync.dma_start(out=kpe_sb, in_=ar_out[0:64])
        kpe_sw = early.tile([64, T], F32, tag="kpe_sw", name="kpe_sw")
        nc.sync.dma_start(out=kpe_sw[0:32], in_=kpe_sb[32:64])
        nc.sync.dma_start(out=kpe_sw[32:64], in_=kpe_sb[0:32])
        kt1 = early.tile([64, T], F32, tag="kt1", name="kt1")
        kt2 = early.tile([64, T], F32, tag="kt2", name="kt2")
        nc.vector.tensor_mul(kt1, kpe_sb, CC[0:64])
        nc.vector.tensor_mul(kt2, kpe_sw, SS[0:64])
        kroped = early.tile([64, T], BF, tag="kroped", name="kroped")
        nc.vector.tensor_add(kroped, kt1, kt2)
        nc.sync.dma_start(out=kroped2[0:64], in_=kroped)
        nc.sync.dma_start(out=kroped2[64:128], in_=kroped)
        es_psA.close()
        es_xpool.close()
        es_xr.close()
        es_early.close()

        # ============ Phase 2: load latents, sumsq, kv_b/q_b ============
        es_heads = ExitStack()
        heads = es_heads.enter_context(tc.tile_pool(name="heads", bufs=1))
        es_qnkv = ExitStack()
        qnkv = es_qnkv.enter_context(tc.tile_pool(name="qnkv", bufs=1))
        es_psQ = ExitStack()
        psQ = es_psQ.enter_context(tc.tile_pool(name="psQ", bufs=1, space="PSUM"))
        es_midkv = ExitStack()
        midkv = es_midkv.enter_context(tc.tile_pool(name="midkv", bufs=1))
        es_sqp = ExitStack()
        sqp = es_sqp.enter_context(tc.tile_pool(name="sqp", bufs=2))

        # AG1 block c = [q rows 192c+128..+191 (64) ; kv rows 64c..+63 (64)]
        # AG2 block c = q rows [192c, 192c+128)
        def q_pieces(k):
            lo, hi = 128 * k, 128 * (k + 1)
            out = []
            for c in range(NC):
                s, e = max(lo, 192 * c), min(hi, 192 * (c + 1))
                if s < e:
                    s2, e2 = s, min(e, 192 * c + 128)
                    if s2 < e2:
                        out.append((s2 - lo, e2 - lo, 2, 128 * c + s2 - 192 * c))
                    s1, e1 = max(s, 192 * c + 128), e
                    if s1 < e1:
                        out.append((s1 - lo, e1 - lo, 1,
                                    128 * c + s1 - 192 * c - 128))
            return out

        def kv_pieces(k):
            lo, hi = 128 * k, 128 * (k + 1)
            out = []
            for c in range(NC):
                s, e = max(lo, 64 * c), min(hi, 64 * (c + 1))
                if s < e:
                    out.append((s - lo, e - lo, 1, 128 * c + 64 + s - 64 * c))
            return out

        def ag_src(which, row, n):
            src = agf1_out if which == 1 else agf2_out
            return src[row:row + n]

        # kv latents first (AG1): sumsq -> inv-rms roundtrip overlaps kn matmuls
        ckv = [qnkv.tile([128, T], BF, tag=f"ckv{k}", name=f"ckv{k}") for k in range(KT_KV)]
        for k in range(KT_KV):
            for (d0, d1, w_, src) in kv_pieces(k):
                nc.sync.dma_start(out=ckv[k][d0:d1], in_=ag_src(w_, src, d1 - d0))
        pskv = [psQ.tile([1, 512], F32, tag=f"dv{c}", name=f"dv{c}") for c in range(2)]
        for k in range(KT_KV):
            sq = sqp.tile([128, T], BF, tag="sq", name="sq")
            nc.vector.tensor_mul(sq, ckv[k], ckv[k])
            for ch in range(2):
                nc.tensor.matmul(pskv[ch], ones_bf,
                                 sq[:, 512 * ch:512 * (ch + 1)],
                                 start=(k == 0), stop=(k == KT_KV - 1))
        invkv_t = midkv.tile([1, T], F32, tag="invkv_t", name="invkv_t")
        tmp2 = midkv.tile([1, T], F32, tag="tmp2", name="tmp2")
        for ch in range(2):
            cs = slice(512 * ch, 512 * (ch + 1))
            nc.scalar.activation(out=tmp2[:, cs], in_=pskv[ch],
                                 func=mybir.ActivationFunctionType.Sqrt,
                                 bias=eps_t, scale=1.0 / RKV)
        nc.vector.reciprocal_approx_fast(out=invkv_t, in_=tmp2)
        invkv_d = dram.tile([1, T], F32, tag="invkv_d", name="invkv_d")
        nc.sync.dma_start(out=invkv_d[:], in_=invkv_t)
        invkv_b = midkv.tile([128, 1, T], F32, tag="invkv_b", name="invkv_b")
        nc.sync.dma_start(out=invkv_b, in_=invkv_d[:].partition_broadcast(128))

        # ============ kv_b projections ============
        kn = [heads.tile([128, T], BF, tag=f"kn{m}", name=f"kn{m}") for m in range(HC)]
        v_t = [heads.tile([128, HC * DV], BF, tag=f"v{t_}", name=f"v{t_}") for t_ in range(8)]

        es_s5 = ExitStack()
        s5 = es_s5.enter_context(tc.tile_pool(name="s5", bufs=2))
        es_s5v = ExitStack()
        s5v = es_s5v.enter_context(tc.tile_pool(name="s5v", bufs=1))
        es_ps5 = ExitStack()
        ps5 = es_ps5.enter_context(tc.tile_pool(name="ps5", bufs=4, space="PSUM"))
        # k_nope on RAW latents (starts immediately), scaled at evacuation
        for mt in range(HC):
            wt = s5.tile([128, KT_KV, 128], BF, tag="wkn", name="wkn")
            nc.sync.dma_start(out=wt, in_=wkn_in[:, mt])
            ps = [ps5.tile([128, 512], F32, tag="ps", name="ps") for _ in range(2)]
            for kt in range(KT_KV):
                for ch in range(2):
                    nc.tensor.matmul(ps[ch], wt[:, kt, :],
                                     ckv[kt][:, 512 * ch:512 * (ch + 1)],
                                     start=(kt == 0), stop=(kt == KT_KV - 1),
                                     skip_group_check=True)
            for ch in range(2):
                nc.vector.tensor_mul(kn[mt][:, 512 * ch:512 * (ch + 1)],
                                     ps[ch], invkv_b[:, 0, 512 * ch:512 * (ch + 1)])

        # q latents (AG2) + sumsq; inv-rms roundtrip overlaps the v matmuls
        qn = [qnkv.tile([128, T], BF, tag=f"qn{k}", name=f"qn{k}") for k in range(KT_Q)]
        for k in range(KT_Q):
            for (d0, d1, w_, src) in q_pieces(k):
                nc.sync.dma_start(out=qn[k][d0:d1], in_=ag_src(w_, src, d1 - d0))
        psqq = [psQ.tile([1, 512], F32, tag=f"dq{c}", name=f"dq{c}") for c in range(2)]
        for k in range(KT_Q):
            sq = sqp.tile([128, T], BF, tag="sq", name="sq")
            nc.vector.tensor_mul(sq, qn[k], qn[k])
            for ch in range(2):
                nc.tensor.matmul(psqq[ch], ones_bf,
                                 sq[:, 512 * ch:512 * (ch + 1)],
                                 start=(k == 0), stop=(k == KT_Q - 1))
        invq_t = midkv.tile([1, T], F32, tag="invq_t", name="invq_t")
        tmp1 = midkv.tile([1, T], F32, tag="tmp1", name="tmp1")
        for ch in range(2):
            cs = slice(512 * ch, 512 * (ch + 1))
            nc.scalar.activation(out=tmp1[:, cs], in_=psqq[ch],
                                 func=mybir.ActivationFunctionType.Sqrt,
                                 bias=eps_t, scale=1.0 / RQ)
        nc.vector.reciprocal_approx_fast(out=invq_t, in_=tmp1)
        invq_d = dram.tile([1, T], F32, tag="invq_d", name="invq_d")
        nc.sync.dma_start(out=invq_d[:], in_=invq_t)
        invq_b = midkv.tile([128, 1, T], F32, tag="invq_b", name="invq_b")
        nc.sync.dma_start(out=invq_b, in_=invq_d[:].partition_broadcast(128))

        # normalize latents in place (overlaps the kn/v matmul streams)
        for k in range(KT_KV):
            nc.vector.tensor_mul(ckv[k], ckv[k], invkv_b[:, 0, :])
        for k in range(KT_Q):
            nc.vector.tensor_mul(qn[k], qn[k], invq_b[:, 0, :])
        # v on normalized latents
        wv_t = s5v.tile([128, KT_KV, HC * DV], BF, tag="wv", name="wv")
        for kt in range(KT_KV):
            nc.sync.dma_start(out=wv_t[:, kt], in_=wv_in[:, kt])
        for tt in range(8):
            pv = [ps5.tile([128, 512], F32, tag="ps", name="ps") for _ in range(4)]
            for kt in range(KT_KV):
                for ch in range(4):
                    nc.tensor.matmul(
                        pv[ch], ckv[kt][:, 128 * tt:128 * (tt + 1)],
                        wv_t[:, kt, 512 * ch:512 * (ch + 1)],
                        start=(kt == 0), stop=(kt == KT_KV - 1),
                        skip_group_check=True)
            for ch in range(4):
                nc.scalar.copy(v_t[tt][:, 512 * ch:512 * (ch + 1)], pv[ch])
        es_ps5.close()
        es_s5v.close()
        es_s5.close()
        es_sqp.close()
        es_midkv.close()
        es_psQ.close()

        # ============ q_b projection + RoPE (normalized latents) ============
        qh = [heads.tile([128, T], BF, tag=f"qh{m}", name=f"qh{m}") for m in range(HC)]
        qr = [heads.tile([128, T], BF, tag=f"qr{m}", name=f"qr{m}") for m in range(8)]

        es_s4 = ExitStack()
        s4 = es_s4.enter_context(tc.tile_pool(name="s4", bufs=3))
        es_s4t = ExitStack()
        s4t = es_s4t.enter_context(tc.tile_pool(name="s4t", bufs=2))
        es_ps4 = ExitStack()
        ps4 = es_ps4.enter_context(tc.tile_pool(name="ps4", bufs=4, space="PSUM"))

        def qb_tile(mt):
            wt = s4.tile([128, KT_Q, 128], BF, tag="wqb", name="wqb")
            nc.sync.dma_start(out=wt, in_=wqb_in[:, mt])
            pss = [ps4.tile([128, 512], F32, tag="ps", name="ps") for _ in range(2)]
            for kt in range(KT_Q):
                for ch in range(2):
                    nc.tensor.matmul(pss[ch], wt[:, kt, :],
                                     qn[kt][:, 512 * ch:512 * (ch + 1)],
                                     start=(kt == 0), stop=(kt == KT_Q - 1),
                                     skip_group_check=True)
            if mt < HC:
                for ch in range(2):
                    nc.scalar.copy(qh[mt][:, 512 * ch:512 * (ch + 1)], pss[ch])
            else:
                rt = mt - HC
                p_bf = s4t.tile([128, T], BF, tag="p_bf", name="p_bf")
                for ch in range(2):
                    nc.scalar.copy(p_bf[:, 512 * ch:512 * (ch + 1)], pss[ch])
                p_sw = s4t.tile([128, T], BF, tag="p_sw", name="p_sw")
                nc.sync.dma_start(out=p_sw[0:32], in_=p_bf[32:64])
                nc.sync.dma_start(out=p_sw[32:64], in_=p_bf[0:32])
                nc.sync.dma_start(out=p_sw[64:96], in_=p_bf[96:128])
                nc.sync.dma_start(out=p_sw[96:128], in_=p_bf[64:96])
                t1 = s4t.tile([128, T], BF, tag="t1", name="t1")
                t2 = s4t.tile([128, T], BF, tag="t2", name="t2")
                nc.vector.tensor_mul(t1, p_bf, CCb)
                nc.vector.tensor_mul(t2, p_sw, SSb)
                nc.vector.tensor_add(qr[rt], t1, t2)

        # head-pair order: nope(2p), nope(2p+1), rope(p) so attention can
        # start as soon as the first pairs are out
        for p in range(8):
            qb_tile(2 * p)
            qb_tile(2 * p + 1)
            qb_tile(HC + p)
        es_ps4.close()
        es_s4t.close()
        es_s4.close()
        if DEBUG:
            for k in range(KT_KV):
                nc.sync.dma_start(out=dbg_ckv[128 * k:128 * (k + 1)], in_=ckv[k])
            for k in range(KT_Q):
                nc.sync.dma_start(out=dbg_qn[128 * k:128 * (k + 1)], in_=qn[k])
            for nm_, src_ in [(dbg_kn, kn[0]), (dbg_qh, qh[0]),
                              (dbg_qr, qr[0]), (dbg_kp, kroped2)]:
                nc.sync.dma_start(out=nm_, in_=src_)
            nc.sync.dma_start(out=dbg_v, in_=v_t[0])
        es_qnkv.close()

        # ====== Attention (16 heads) with interleaved o_proj sub-passes ======
        es_s6 = ExitStack()
        s6 = es_s6.enter_context(tc.tile_pool(name="s6", bufs=3))
        es_s6b = ExitStack()
        s6b = es_s6b.enter_context(tc.tile_pool(name="s6b", bufs=2))
        es_oacc = ExitStack()
        oaccp = es_oacc.enter_context(tc.tile_pool(name="oacc", bufs=1))
        es_rh = ExitStack()
        rhp = es_rh.enter_context(tc.tile_pool(name="rh", bufs=KTC))
        es_wop = ExitStack()
        wop = es_wop.enter_context(tc.tile_pool(name="wop", bufs=2))
        es_psS = ExitStack()
        psS = es_psS.enter_context(tc.tile_pool(name="psS", bufs=2, space="PSUM"))
        es_psO = ExitStack()
        psO = es_psO.enter_context(tc.tile_pool(name="psO", bufs=1, space="PSUM"))
        es_psD = ExitStack()
        psD = es_psD.enter_context(tc.tile_pool(name="psD", bufs=1, space="PSUM"))
        es_psP = ExitStack()
        psP = es_psP.enter_context(tc.tile_pool(name="psP", bufs=3, space="PSUM"))

        o_acc = [oaccp.tile([128, T], BF, tag=f"oa{m}", name=f"oa{m}")
                 for m in range(7)]

        def attention_head(hh):
            rt, half = hh // 2, hh % 2
            qr_sl = qr[rt][64 * half:64 * (half + 1)]
            kp_sl = kroped2[64 * half:64 * (half + 1)]
            pso = [psO.tile([128, 512], F32, tag=f"o{c}", name=f"o{c}") for c in range(2)]
            acc = s6b.tile([128, T], BF, tag="acc", name="acc")

            def do_av(jt, ets):
                for ch in range(2):
                    if ets[ch] is None:
                        continue
                    et, ns, ne, w = ets[ch]
                    ost = ns - 512 * ch
                    nc.tensor.matmul(pso[ch][:, ost:512],
                                     v_t[jt][:, 128 * hh:128 * (hh + 1)],
                                     et[:, 0:w], start=(jt == 0), stop=(jt == 7),
                                     skip_group_check=True)
                    if jt == 0:
                        nc.vector.tensor_copy(acc[:, ns:ne], et[:, 0:w])
                    else:
                        nc.vector.tensor_add(acc[:, ns:ne], acc[:, ns:ne],
                                             et[:, 0:w])

            pend = None
            for jt in range(8):
                qlo = 128 * jt
                pts = []
                for ch in range(2):
                    ns, ne = max(qlo, 512 * ch), 512 * (ch + 1)
                    if ns >= ne:
                        pts.append(None)
                        continue
                    w = ne - ns
                    pst = psS.tile([128, 512], F32, tag="s", name="s")
                    pts.append((pst, ns, ne, w))
                    nc.tensor.matmul(pst[:, 0:w], kn[hh][:, qlo:qlo + 128],
                                     qh[hh][:, ns:ne], start=True, stop=False,
                                     skip_group_check=True)
                for ch in range(2):
                    if pts[ch] is None:
                        continue
                    pst, ns, ne, w = pts[ch]
                    nc.tensor.matmul(pst[:, 0:w], kp_sl[:, qlo:qlo + 128],
                                     qr_sl[:, ns:ne], start=False, stop=True,
                                     skip_group_check=True)
                # previous chunk's AV goes after this chunk's score matmuls so
                # the PE never waits on the exp
                if pend is not None:
                    do_av(*pend)
                ets = []
                for ch in range(2):
                    if pts[ch] is None:
                        ets.append(None)
                        continue
                    pst, ns, ne, w = pts[ch]
                    et = s6.tile([128, 512], BF, tag="et", name="et")
                    nc.scalar.activation(out=et[:, 0:w], in_=pst[:, 0:w],
                                         func=mybir.ActivationFunctionType.Exp,
                                         scale=SCALE)
                    if ns == qlo:
                        nc.vector.tensor_mul(et[:, 0:128], et[:, 0:128], mask_t)
                    ets.append((et, ns, ne, w))
                pend = (jt, ets)
            do_av(*pend)
            # evacuate AV result to SBUF right away to free the PSUM banks,
            # scale by 1/den later when the broadcast lands
            ob = s6b.tile([128, T], BF, tag="ob", name="ob")
            for ch in range(2):
                cs = slice(512 * ch, 512 * (ch + 1))
                nc.vector.tensor_copy(ob[:, cs], pso[ch])
            rec = s6b.tile([1, T], F32, tag="rec", name="rec")
            rec_bf = s6b.tile([1, T], BF, tag="rec_bf", name="rec_bf")
            den_sb = s6b.tile([1, T], F32, tag="den_sb", name="den_sb")
            for ch in range(2):
                cs = slice(512 * ch, 512 * (ch + 1))
                psd = psD.tile([1, 512], F32, tag="d", name="d")
                nc.tensor.matmul(psd, ones_bf, acc[:, cs], start=True, stop=True,
                                 skip_group_check=True)
                nc.scalar.copy(den_sb[:, cs], psd)
            nc.vector.reciprocal_approx_fast(out=rec, in_=den_sb)
            nc.vector.tensor_copy(rec_bf, rec)
            nc.sync.dma_start(out=db_d[hh:hh + 1], in_=rec_bf)
            den_b = s6b.tile([128, 1, T], BF, tag="den_b", name="den_b")
            nc.sync.dma_start(out=den_b,
                              in_=db_d[hh:hh + 1].partition_broadcast(128))
            nc.vector.tensor_mul(ob, ob, den_b[:, 0, :])
            nc.sync.dma_start(out=ago_in[hh], in_=ob)
            if DEBUG:
                nc.sync.dma_start(out=dbg_ob[128 * hh:128 * (hh + 1)], in_=ob)
                if hh == 0:
                    nc.sync.dma_start(out=dbg_acc, in_=acc)

        def oproj_subpass(j):
            # k-tiles j*KTC .. j*KTC+KTC-1 of the 128-k-tile o_proj sum
            rhs = []
            for kt in range(KTC):
                rh = rhp.tile([128, T], BF, tag="rh", name="rh")
                nc.sync.dma_start(out=rh, in_=ago_out[j][128 * kt:128 * (kt + 1)])
                rhs.append(rh)
            for m in range(7):
                wt = wop.tile([128, KTC, 128], BF, tag="wo", name="wo")
                nc.sync.dma_start(out=wt, in_=wo_in[:, m, KTC * j:KTC * (j + 1)])
                po = [psP.tile([128, 512], F32, tag="po", name="po") for _ in range(2)]
                for kt in range(KTC):
                    for ch in range(2):
                        nc.tensor.matmul(po[ch], wt[:, kt, :],
                                         rhs[kt][:, 512 * ch:512 * (ch + 1)],
                                         start=(kt == 0), stop=(kt == KTC - 1),
                                         skip_group_check=True)
                for ch in range(2):
                    cs = slice(512 * ch, 512 * (ch + 1))
                    if j == 0:
                        nc.vector.tensor_copy(o_acc[m][:, cs], po[ch])
                    else:
                        nc.vector.tensor_add(o_acc[m][:, cs], o_acc[m][:, cs],
                                             po[ch])
                if j == NAG - 1:
                    nc.sync.dma_start(out=out_ap[128 * m:128 * (m + 1)],
                                      in_=o_acc[m])

        # lag-2 sub-pass schedule at the start (AG latency cover), catch up to
        # lag-1 at hh=8 so the tail is a single sub-pass
        for hh in range(HC):
            attention_head(hh)
            nc.gpsimd.collective_compute(
                "AllGather", mybir.AluOpType.bypass, replica_groups=RG,
                ins=[ago_in[hh].opt()], outs=[ago_out[hh].opt()])
            if 2 <= hh <= 7:
                oproj_subpass(hh - 2)
            elif hh == 8:
                oproj_subpass(6)
                oproj_subpass(7)
            elif hh >= 9:
                oproj_subpass(hh - 1)
        oproj_subpass(NAG - 1)

        es_psP.close()
        es_psD.close()
        es_psO.close()
        es_psS.close()
        es_wop.close()
        es_rh.close()
        es_oacc.close()
        es_s6b.close()
        es_s6.close()
        es_heads.close()
        es_persist.close()
        es_dram.close()

    nc.finalize()
    return nc


def _prep_inputs(hidden_states, cos, sin, wq_a, q_ln_w, wq_b, wkv_a, kv_ln_w,
                 wkv_b, wo):
    """Host-side sharding + layout. Returns in_maps (list of dicts per core)."""
    h2 = np.ascontiguousarray(hidden_states.reshape(S, D).T)      # [D, T]
    xh = np.ascontiguousarray(
        h2.reshape(KT_X, 128, T).transpose(1, 0, 2)).astype(bf16)  # [128,56,T]

    cosT = np.ascontiguousarray(cos.reshape(T, DR).T).astype(np.float32)
    sinT = np.ascontiguousarray(sin.reshape(T, DR).T).astype(np.float32)
    CCh = np.vstack([cosT, cosT])
    SSh = np.vstack([-sinT[:32], sinT[32:], -sinT[:32], sinT[32:]])
    ccss = np.ascontiguousarray(np.hstack([CCh, SSh])).astype(np.float32)

    wq_b_eff = (wq_b * q_ln_w[None, :]).astype(np.float32)
    wkv_b_eff = (wkv_b * kv_ln_w[None, :]).astype(np.float32)
    wq_b_r = wq_b_eff.reshape(H, DQ, RQ)
    wkv_b_r = wkv_b_eff.reshape(H, DN + DV, RKV)
    woT = wo.T                                                    # [16384, D]

    def lhst_tiles(lhsT, kt, mt):
        # [K, M] -> [128, mt, kt, 128]
        K, M = lhsT.shape
        return np.ascontiguousarray(
            lhsT.reshape(kt, 128, mt, 128).transpose(1, 2, 0, 3)).astype(bf16)

    in_maps = []
    for c in range(NC):
        m = {"x": xh, "ccss": ccss}
        m["xkpe"] = np.ascontiguousarray(xh[:, 7 * c:7 * (c + 1), :])
        # stage A slice: 192 q rows + 64 kv rows  -> lhsT [D, 256]
        qs = wq_a[192 * c:192 * (c + 1)]                           # [192, D]
        ks = wkv_a[64 * c:64 * (c + 1)]                            # [64, D]
        lhsT_a = np.vstack([qs, ks]).T                             # [D, 256]
        m["wa"] = lhst_tiles(lhsT_a, KT_X, 2).transpose(0, 2, 1, 3).copy()
        # k_pe K-shard: wkv_a rows 512:576, K cols 896c..
        lhsT_kpe = wkv_a[RKV:RKV + DR, 896 * c:896 * (c + 1)].T    # [896, 64]
        m["wkpe"] = np.ascontiguousarray(
            lhsT_kpe.reshape(7, 128, DR).transpose(1, 0, 2)).astype(bf16)
        hs = slice(HC * c, HC * (c + 1))
        wq_b_c = wq_b_r[hs]                                        # [16,192,RQ]
        lhsT_qb = np.vstack([
            wq_b_c[:, :DN, :].reshape(HC * DN, RQ),
            wq_b_c[:, DN:, :].reshape(HC * DR, RQ)]).T             # [RQ, 3072]
        m["wqb"] = lhst_tiles(lhsT_qb, KT_Q, MT_QB)
        lhsT_kn = wkv_b_r[hs][:, :DN, :].reshape(HC * DN, RKV).T   # [RKV, 2048]
        m["wkn"] = lhst_tiles(lhsT_kn, KT_KV, HC)
        rhs_v = wkv_b_r[hs][:, DN:, :].reshape(HC * DV, RKV).T     # [RKV, 2048]
        m["wv"] = np.ascontiguousarray(
            rhs_v.reshape(KT_KV, 128, HC * DV).transpose(1, 0, 2)).astype(bf16)
        # permute wo k-tiles (= heads) into AG-chunk order: chunk j's k-tile
        # kt holds global head 16*(kt//HPC) + HPC*j + kt%HPC
        perm = [16 * (kt // HPC) + HPC * j + kt % HPC
                for j in range(NAG) for kt in range(KTC)]
        woT_p = woT.reshape(H, DV, D)[perm].reshape(H * DV, D)
        lhsT_wo = woT_p[:, 896 * c:896 * (c + 1)]                  # [16384, 896]
        m["wo"] = lhst_tiles(lhsT_wo, H * DV // 128, 7)            # [128,7,128,128]
        in_maps.append(m)
    return in_maps


def _get_nc():
    if "nc" not in _CACHE:
        _CACHE["nc"] = _build()
    return _CACHE["nc"]


def run(in_maps, trace=False, trace_kwargs=None):
    nc = _get_nc()
    return run_bass_kernel_spmd(nc, in_maps, list(range(NC)), trace=trace,
                                **(trace_kwargs or {}))


def kernel(hidden_states, cos, sin, wq_a, q_ln_w, wq_b, wkv_a, kv_ln_w,
           wkv_b, wo):
    in_maps = _prep_inputs(hidden_states, cos, sin, wq_a, q_ln_w, wq_b,
                           wkv_a, kv_ln_w, wkv_b, wo)
    res = run(in_maps)
    out = np.concatenate([res.results[c]["out"] for c in range(NC)], axis=0)
    return np.ascontiguousarray(out.T).reshape(B, S, D).astype(np.float32)
